# revision 17
# baseline (speedup 1.0000x reference)
"""Grouped attention pooling kernel for Trainium2 (8 NeuronCores, SPMD).

Reference computation (T=2048 agents, 128 sorted groups, d=64):
    Wh = h @ W.T + b
    sigma[i,j] = f[i,j,:] . Wh[j,:]
    scores     = sigma masked to the query's group (self -> -1000, outside -> -inf)
    attn       = softmax(scores, axis=1);  S = attn @ h;  size-1 groups -> 0

segment_ids is sorted, so attention is block-diagonal over groups (mean size
~16): only f[i, lo_g:hi_g, :] is ever needed (~9 MB of the 1 GiB tensor).
The host packs those blocks into per-group 32-row "slots"; groups are
sharded across the 8 cores (data parallel, no cross-device attention).
Every core runs one identical program; only the packed data differs.
Groups are assigned to (core, slot) by descending size in a boustrophedon
stripe, so tile t on every core only holds groups of size <= K_t =
sizes_sorted[32*t]; tile t's multiply/reduce/DMA free width is trimmed to
K_t*64.

f blocks are packed TRANSPOSED (keys on partitions, (query, d) along free)
so the Wh operand of the sigma multiply is the per-(slot,key) Wh row tile
broadcast along the free dim with a stride-0 access pattern — the big Wh
replication needs no DMA and no SBUF copy. Wh itself is computed directly
in [(slot,k), d] layout by per-tile PE matmuls ([hT|1]^T @ [W^T|b]).

Per-core device program:
  1. whp[(slot,k), d] = hkt_aug[:,tile]^T @ wt_aug   (one PE matmul per tile)
  2. per 128-row tile: fpackT * broadcast(whp) (GpSimd/DVE), segmented
     d-reduce (DVE) -> sigmaT[k, q]; DVE 32x32 block transpose -> sigma[q, k]
  3. additive mask, softmax on DVE/ACT (exp sum fused via accum_out;
     1/sum folded into the output copy's per-partition scale)
  4. per-slot attn^T (DVE block transpose) @ hkey -> S (PE 32x32
     tile_position blocks), DMA out
"""
import sys
import types
import numpy as np
from contextlib import ExitStack

try:  # keep run_bass_kernel_spmd's BASS_TRACE path from crashing when the
    import antenv.axon_hooks  # noqa: F401  # image lacks the axon NTFF hook
except Exception:
    _m = types.ModuleType("antenv.axon_hooks")
    _m.get_axon_ntff_profile_hook = lambda: None
    _m.set_axon_ntff_profile_hook = lambda h: None
    sys.modules.setdefault("antenv.axon_hooks", _m)

import concourse.bass as bass
import concourse.bacc as bacc
import concourse.tile as tile
import concourse.mybir as mybir
from concourse.bass_utils import run_bass_kernel_spmd
from bass_rust import AxisListType

N_CORES = 8
D = 64
NEG = -1.0e30
SELF_MASK = -1000.0
F32 = mybir.dt.float32

LAST_RESULT = None  # BassKernelResults of the most recent run (for test harness)
_PROGRAM_CACHE = {}

# engine for the big per-tile multiply, indexed by tile (tunable); the DVE
# pays ~2x on stride-0-broadcast operands, so GpSimd wins for all of these
MUL_ENGINE = ["gpsimd", "gpsimd", "gpsimd", "gpsimd"]


def _build_program(K_pad: int, rows: int, K_tile: tuple):
    """One SPMD program, identical across cores. rows = padded rows/core."""
    spt = 128 // K_pad          # slots per 128-row tile
    n_tiles = rows // 128

    nc = bacc.Bacc("TRN2", target_bir_lowering=False, debug=False,
                   enable_asserts=True, num_devices=N_CORES)

    fpackt = nc.dram_tensor("fpackt", [rows, K_pad * D], F32, kind="ExternalInput")
    hkey = nc.dram_tensor("hkey", [rows, D], F32, kind="ExternalInput")
    hkt_aug = nc.dram_tensor("hkt_aug", [D + 1, rows], F32, kind="ExternalInput")
    wt_aug = nc.dram_tensor("wt_aug", [D + 1, D], F32, kind="ExternalInput")
    m0 = nc.dram_tensor("m0", [rows, K_pad], F32, kind="ExternalInput")
    ident_in = nc.dram_tensor("ident", [64, 64], F32, kind="ExternalInput")
    out = nc.dram_tensor("out", [rows, D], F32, kind="ExternalOutput")

    with tile.TileContext(nc) as tc, ExitStack() as ctx:
        const = ctx.enter_context(tc.tile_pool(name="const", bufs=1))
        small = ctx.enter_context(tc.tile_pool(name="small", bufs=3))
        ldp = ctx.enter_context(tc.tile_pool(name="ldp", bufs=n_tiles))
        big = ctx.enter_context(tc.tile_pool(name="big", bufs=2))
        ps = ctx.enter_context(tc.tile_pool(name="ps", bufs=2, space="PSUM"))

        # ---- tiny const loads first (sync) so the Wh matmuls are not
        # stuck behind the megabyte fpackt stream; bulk loads on scalar ----
        wt_t = const.tile([D + 1, D], F32)
        nc.sync.dma_start(wt_t[:], wt_aug[:])
        hkt_t = const.tile([D + 1, rows], F32)
        nc.sync.dma_start(hkt_t[:], hkt_aug[:])
        ident = const.tile([64, 64], F32)
        if K_pad == 64:
            nc.sync.dma_start(ident[:], ident_in[:])
        fts = []
        for t in range(n_tiles):
            ft = ldp.tile([128, K_pad * D], F32, tag="ft")
            nc.sync.dma_start(ft[:, :K_tile[t] * D],
                              fpackt[t * 128:t * 128 + 128, :K_tile[t] * D])
            fts.append(ft)
        m0s, hks = [], []
        for t in range(n_tiles):
            r0 = t * 128
            m0_t = ldp.tile([128, K_pad], F32, tag="m0_t")
            nc.scalar.dma_start(m0_t[:], m0[r0:r0 + 128, :])
            hk_t = ldp.tile([128, D], F32, tag="hk_t")
            nc.scalar.dma_start(hk_t[:], hkey[r0:r0 + 128, :])
            m0s.append(m0_t)
            hks.append(hk_t)

        # ---- Wh rows in [(slot,k), d] layout: one matmul per tile.
        # One SBUF tile per query-tile: tile t's multiply then only waits
        # for its own Wh slice, not all of them. ----
        whp_sbs = []
        for t in range(n_tiles):
            whp_ps = ps.tile([128, D], F32, tag="whp_ps")
            nc.tensor.matmul(whp_ps[:], hkt_t[:, t * 128:(t + 1) * 128],
                             wt_t[:], start=True, stop=True)
            whp_sb = const.tile([128, D], F32, tag=f"whp_sb{t}")
            nc.scalar.activation(whp_sb[:], whp_ps[:],
                                 mybir.ActivationFunctionType.Identity)
            whp_sbs.append(whp_sb)

        # ---------- per 128-row tile ----------
        for t in range(n_tiles):
            r0 = t * 128
            Kt = K_tile[t]
            FT = Kt * D
            ft, m0_t, hk_t = fts[t], m0s[t], hks[t]

            # sigmaT[k, q] = sum_d fT[k, (q,d)] * Wh[(slot,k), d]
            prod = big.tile([128, K_pad * D], F32, tag="prod")
            whb = whp_sbs[t][:].unsqueeze(1).broadcast_to((128, Kt, D))
            mul_eng = getattr(nc, MUL_ENGINE[t % len(MUL_ENGINE)])
            mul_eng.tensor_mul(prod[:, :FT].rearrange("p (q d) -> p q d", d=D),
                               ft[:, :FT].rearrange("p (q d) -> p q d", d=D),
                               whb)
            sigT = small.tile([128, K_pad], F32, tag="sigT")
            if Kt < K_pad:
                nc.vector.memset(sigT[:], 0.0)  # stale cols would poison rows
            nc.vector.tensor_reduce(
                sigT[:, :Kt].unsqueeze(2),
                prod[:, :FT].rearrange("p (q d) -> p q d", d=D),
                axis=AxisListType.X, op=mybir.AluOpType.add)

            sig = small.tile([128, K_pad], F32, tag="sig")
            nc.vector.transpose(sig[:], sigT[:])

            scores = small.tile([128, K_pad], F32, tag="scores")
            nc.vector.tensor_add(scores[:], sig[:], m0_t[:])

            negmax = small.tile([128, 1], F32, tag="negmax")
            nc.vector.tensor_reduce(negmax[:], scores[:], axis=AxisListType.X,
                                    op=mybir.AluOpType.max, negate=True)
            exps = small.tile([128, K_pad], F32, tag="exps")
            sumexp = small.tile([128, 1], F32, tag="sumexp")
            nc.scalar.activation(exps[:], scores[:],
                                 mybir.ActivationFunctionType.Exp,
                                 bias=negmax[:], scale=1.0, accum_out=sumexp[:])
            rinv = small.tile([128, 1], F32, tag="rinv")
            nc.vector.reciprocal(rinv[:], sumexp[:])
            attn = exps  # unnormalized; 1/sumexp folded into the S copy below

            s_ps = ps.tile([128, D], F32, tag="s_ps")
            if K_pad == 32:
                attnT = small.tile([128, K_pad], F32, tag="attnT")
                nc.vector.transpose(attnT[:], attn[:])
                for j in range(4):
                    sl = slice(32 * j, 32 * j + 32)
                    nc.tensor.matmul(s_ps[sl, :], attnT[sl, :], hk_t[sl, :],
                                     start=True, stop=True,
                                     tile_position=(32 * j, 32 * j))
            else:  # K_pad == 64: PE transpose per slot
                for j in range(spt):
                    sl = slice(64 * j, 64 * j + 64)
                    aT_ps = ps.tile([64, 64], F32, tag="aT_ps")
                    nc.tensor.transpose(aT_ps[:], attn[sl, :], ident[:],
                                        tile_position=(64 * j, 0))
                    aT_sb = small.tile([64, 64], F32, tag="aT_sb")
                    nc.scalar.activation(aT_sb[:], aT_ps[:],
                                         mybir.ActivationFunctionType.Identity)
                    nc.tensor.matmul(s_ps[sl, :], aT_sb[:], hk_t[sl, :],
                                     start=True, stop=True,
                                     tile_position=(0, 64 * j))

            s_sb = small.tile([128, D], F32, tag="s_sb")
            nc.scalar.activation(s_sb[:], s_ps[:],
                                 mybir.ActivationFunctionType.Identity,
                                 scale=rinv[:])
            nc.sync.dma_start(out[r0:r0 + 128, :], s_sb[:])

    nc.compile()
    return nc


def _plan(seg):
    T = seg.shape[0]
    change = np.nonzero(np.diff(seg))[0] + 1
    starts = np.concatenate([[0], change]).astype(np.int64)
    ends = np.concatenate([change, [T]]).astype(np.int64)
    sizes = ends - starts
    smax = int(sizes.max())
    if smax <= 32:
        K_pad = 32
    elif smax <= 64:
        K_pad = 64
    else:
        raise NotImplementedError(f"group size {smax} > 64")
    G = len(starts)
    S_dev = -(-G // N_CORES)
    rows = -(-(S_dev * K_pad) // 128) * 128
    spt = 128 // K_pad
    n_tiles = rows // 128

    # size-descending boustrophedon assignment: rank r -> core, slot r//8
    order = np.argsort(-sizes, kind="stable")          # group ids by size desc
    assign = {}                                        # g -> (core, slot)
    for r, g in enumerate(order):
        j = r // N_CORES
        c = r % N_CORES if j % 2 == 0 else N_CORES - 1 - (r % N_CORES)
        assign[int(g)] = (c, j)
    sizes_desc = sizes[order]
    K_tile = []
    for t in range(n_tiles):
        r = t * spt * N_CORES
        K_tile.append(int(sizes_desc[r]) if r < G else 1)
    return starts, ends, sizes, G, K_pad, S_dev, rows, assign, tuple(K_tile)


def _pack(f, h, seg, W, b):
    starts, ends, sizes, G, K_pad, S_dev, rows, assign, K_tile = _plan(seg)
    wt_aug = np.concatenate([W.T, b[None, :]], axis=0)  # [65, 64]
    ident = np.eye(64, dtype=np.float32)

    fpackt = np.zeros((N_CORES, rows, K_pad * D), dtype=np.float32)
    hkey = np.zeros((N_CORES, rows, D), dtype=np.float32)
    hkt_aug = np.zeros((N_CORES, D + 1, rows), dtype=np.float32)
    hkt_aug[:, D, :] = 1.0
    m0 = np.full((N_CORES, rows, K_pad), NEG, dtype=np.float32)
    for g in range(G):
        c, j = assign[g]
        lo, hi, s = starts[g], ends[g], int(sizes[g])
        r = j * K_pad
        blk = f[lo:hi, lo:hi, :]                      # [q, k, d]
        fpackt[c, r:r + s, :s * D] = blk.transpose(1, 0, 2).reshape(s, s * D)
        hkey[c, r:r + s, :] = h[lo:hi, :]
        hkt_aug[c, :D, r:r + s] = h[lo:hi, :].T
        m0[c, r:r + s, :s] = 0.0
        m0[c, np.arange(r, r + s), np.arange(s)] = SELF_MASK
    in_maps = [{"fpackt": fpackt[c], "hkey": hkey[c], "hkt_aug": hkt_aug[c],
                "wt_aug": wt_aug, "m0": m0[c], "ident": ident}
               for c in range(N_CORES)]
    meta = (starts, ends, sizes, G, K_pad, S_dev, rows, assign, K_tile)
    return in_maps, meta


def _unpack(per_core_out, meta, T):
    starts, ends, sizes, G, K_pad, S_dev, rows, assign, K_tile = meta
    outf = np.zeros((T, D), dtype=np.float32)
    for g in range(G):
        c, j = assign[g]
        if sizes[g] > 1:
            outf[starts[g]:ends[g], :] = \
                per_core_out[c][j * K_pad:j * K_pad + int(sizes[g]), :]
    return outf


def kernel(f, h, segment_ids, W, b):
    global LAST_RESULT
    f = np.asarray(f, dtype=np.float32)
    h = np.asarray(h, dtype=np.float32)
    seg = np.asarray(segment_ids)
    W = np.asarray(W, dtype=np.float32)
    b = np.asarray(b, dtype=np.float32)
    T = h.shape[0]

    in_maps, meta = _pack(f, h, seg, W, b)
    K_pad, rows, K_tile = meta[4], meta[6], meta[8]

    key = (K_pad, rows, K_tile)
    if key not in _PROGRAM_CACHE:
        _PROGRAM_CACHE[key] = _build_program(K_pad, rows, K_tile)
    nc = _PROGRAM_CACHE[key]

    res = run_bass_kernel_spmd(nc, in_maps, core_ids=list(range(N_CORES)))
    LAST_RESULT = res
    return _unpack([res.results[dev]["out"] for dev in range(N_CORES)], meta, T)


# revision 19
# speedup vs baseline: 1.1675x; 1.1675x over previous
"""Grouped attention pooling kernel for Trainium2 (8 NeuronCores, SPMD).

Reference computation (T=2048 agents, 128 sorted groups, d=64):
    Wh = h @ W.T + b
    sigma[i,j] = f[i,j,:] . Wh[j,:]
    scores     = sigma masked to the query's group (self -> -1000, outside -> -inf)
    attn       = softmax(scores, axis=1);  S = attn @ h;  size-1 groups -> 0

segment_ids is sorted, so attention is block-diagonal over groups (mean size
~16): only f[i, lo_g:hi_g, :] is ever needed (~9 MB of the 1 GiB tensor).
The host packs those blocks into per-group 32-row "slots"; groups are
sharded across the 8 cores (data parallel, no cross-device attention).
Every core runs one identical program; only the packed data differs.
Groups are assigned to (core, slot) by descending size in a boustrophedon
stripe, so tile t on every core only holds groups of size <= K_t =
sizes_sorted[32*t]; tile t's multiply/reduce/DMA free width is trimmed to
K_t*64.

f blocks are packed TRANSPOSED (keys on partitions, (query, d) along free)
so the Wh operand of the sigma multiply is the per-(slot,key) Wh row tile
broadcast along the free dim with a stride-0 access pattern — the big Wh
replication needs no DMA and no SBUF copy. Wh itself is computed directly
in [(slot,k), d] layout by per-tile PE matmuls ([hT|1]^T @ [W^T|b]).

Per-core device program:
  1. whp[(slot,k), d] = hkt_aug[:,tile]^T @ wt_aug   (one PE matmul per tile)
  2. per 128-row tile: fpackT * broadcast(whp) (GpSimd/DVE), segmented
     d-reduce (DVE) -> sigmaT[k, q]; DVE 32x32 block transpose -> sigma[q, k]
  3. additive mask, softmax on DVE/ACT (exp sum fused via accum_out;
     1/sum folded into the output copy's per-partition scale)
  4. per-slot attn^T (DVE block transpose) @ hkey -> S (PE 32x32
     tile_position blocks), DMA out
"""
import sys
import types
import numpy as np
from contextlib import ExitStack

try:  # keep run_bass_kernel_spmd's BASS_TRACE path from crashing when the
    import antenv.axon_hooks  # noqa: F401  # image lacks the axon NTFF hook
except Exception:
    _m = types.ModuleType("antenv.axon_hooks")
    _m.get_axon_ntff_profile_hook = lambda: None
    _m.set_axon_ntff_profile_hook = lambda h: None
    sys.modules.setdefault("antenv.axon_hooks", _m)

import concourse.bass as bass
import concourse.bacc as bacc
import concourse.tile as tile
import concourse.mybir as mybir
from concourse.bass_utils import run_bass_kernel_spmd
from bass_rust import AxisListType

N_CORES = 8
D = 64
NEG = -1.0e30
SELF_MASK = -1000.0
F32 = mybir.dt.float32

LAST_RESULT = None  # BassKernelResults of the most recent run (for test harness)
_PROGRAM_CACHE = {}

# engine for the big per-tile multiply, indexed by tile (tunable); the DVE
# pays ~2x on stride-0-broadcast operands, so GpSimd wins for all of these
MUL_ENGINE = ["gpsimd", "gpsimd", "gpsimd", "vector"]


def _build_program(K_pad: int, rows: int, K_tile: tuple):
    """One SPMD program, identical across cores. rows = padded rows/core."""
    spt = 128 // K_pad          # slots per 128-row tile
    n_tiles = rows // 128

    nc = bacc.Bacc("TRN2", target_bir_lowering=False, debug=False,
                   enable_asserts=True, num_devices=N_CORES)

    fpackt = nc.dram_tensor("fpackt", [rows, K_pad * D], F32, kind="ExternalInput")
    hkey = nc.dram_tensor("hkey", [rows, D], F32, kind="ExternalInput")
    hkt_aug = nc.dram_tensor("hkt_aug", [D + 1, rows], F32, kind="ExternalInput")
    wt_aug = nc.dram_tensor("wt_aug", [D + 1, D], F32, kind="ExternalInput")
    m0 = nc.dram_tensor("m0", [rows, K_pad], F32, kind="ExternalInput")
    ident_in = nc.dram_tensor("ident", [64, 64], F32, kind="ExternalInput")
    out = nc.dram_tensor("out", [rows, D], F32, kind="ExternalOutput")

    with tile.TileContext(nc) as tc, ExitStack() as ctx:
        const = ctx.enter_context(tc.tile_pool(name="const", bufs=1))
        small = ctx.enter_context(tc.tile_pool(name="small", bufs=3))
        ldp = ctx.enter_context(tc.tile_pool(name="ldp", bufs=n_tiles))
        big = ctx.enter_context(tc.tile_pool(name="big", bufs=2))
        ps = ctx.enter_context(tc.tile_pool(name="ps", bufs=2, space="PSUM"))

        # ---- tiny const loads first (sync) so the Wh matmuls are not
        # stuck behind the megabyte fpackt stream; bulk loads on scalar ----
        wt_t = const.tile([D + 1, D], F32)
        nc.sync.dma_start(wt_t[:], wt_aug[:])
        hkt_t = const.tile([D + 1, rows], F32)
        nc.sync.dma_start(hkt_t[:], hkt_aug[:])
        ident = const.tile([64, 64], F32)
        if K_pad == 64:
            nc.sync.dma_start(ident[:], ident_in[:])
        fts = []
        for t in range(n_tiles):
            ft = ldp.tile([128, K_pad * D], F32, tag="ft")
            nc.sync.dma_start(ft[:, :K_tile[t] * D],
                              fpackt[t * 128:t * 128 + 128, :K_tile[t] * D])
            fts.append(ft)
        m0s, hks = [], []
        for t in range(n_tiles):
            r0 = t * 128
            m0_t = ldp.tile([128, K_pad], F32, tag="m0_t")
            nc.scalar.dma_start(m0_t[:], m0[r0:r0 + 128, :])
            hk_t = ldp.tile([128, D], F32, tag="hk_t")
            nc.scalar.dma_start(hk_t[:], hkey[r0:r0 + 128, :])
            m0s.append(m0_t)
            hks.append(hk_t)

        # ---- Wh rows in [(slot,k), d] layout: one matmul per tile ----
        whp_sb = const.tile([128, n_tiles * D], F32)
        for t in range(n_tiles):
            whp_ps = ps.tile([128, D], F32, tag="whp_ps")
            nc.tensor.matmul(whp_ps[:], hkt_t[:, t * 128:(t + 1) * 128],
                             wt_t[:], start=True, stop=True)
            nc.scalar.activation(whp_sb[:, t * D:(t + 1) * D], whp_ps[:],
                                 mybir.ActivationFunctionType.Identity)

        # ---------- per 128-row tile ----------
        for t in range(n_tiles):
            r0 = t * 128
            Kt = K_tile[t]
            FT = Kt * D
            ft, m0_t, hk_t = fts[t], m0s[t], hks[t]

            # sigmaT[k, q] = sum_d fT[k, (q,d)] * Wh[(slot,k), d]
            prod = big.tile([128, K_pad * D], F32, tag="prod")
            whb = whp_sb[:, t * D:(t + 1) * D].unsqueeze(1) \
                .broadcast_to((128, Kt, D))
            mul_eng = getattr(nc, MUL_ENGINE[t % len(MUL_ENGINE)])
            mul_eng.tensor_mul(prod[:, :FT].rearrange("p (q d) -> p q d", d=D),
                               ft[:, :FT].rearrange("p (q d) -> p q d", d=D),
                               whb)
            sigT = small.tile([128, K_pad], F32, tag="sigT")
            if Kt < K_pad:
                nc.vector.memset(sigT[:], 0.0)  # stale cols would poison rows
            nc.vector.tensor_reduce(
                sigT[:, :Kt].unsqueeze(2),
                prod[:, :FT].rearrange("p (q d) -> p q d", d=D),
                axis=AxisListType.X, op=mybir.AluOpType.add)

            sig = small.tile([128, K_pad], F32, tag="sig")
            nc.vector.transpose(sig[:], sigT[:])

            scores = small.tile([128, K_pad], F32, tag="scores")
            nc.vector.tensor_add(scores[:], sig[:], m0_t[:])

            negmax = small.tile([128, 1], F32, tag="negmax")
            nc.vector.tensor_reduce(negmax[:], scores[:], axis=AxisListType.X,
                                    op=mybir.AluOpType.max, negate=True)
            exps = small.tile([128, K_pad], F32, tag="exps")
            sumexp = small.tile([128, 1], F32, tag="sumexp")
            nc.scalar.activation(exps[:], scores[:],
                                 mybir.ActivationFunctionType.Exp,
                                 bias=negmax[:], scale=1.0, accum_out=sumexp[:])
            rinv = small.tile([128, 1], F32, tag="rinv")
            nc.vector.reciprocal(rinv[:], sumexp[:])
            attn = exps  # unnormalized; 1/sumexp folded into the S copy below

            s_ps = ps.tile([128, D], F32, tag="s_ps")
            if K_pad == 32:
                attnT = small.tile([128, K_pad], F32, tag="attnT")
                nc.vector.transpose(attnT[:], attn[:])
                for j in range(4):
                    sl = slice(32 * j, 32 * j + 32)
                    nc.tensor.matmul(s_ps[sl, :], attnT[sl, :], hk_t[sl, :],
                                     start=True, stop=True,
                                     tile_position=(32 * j, 32 * j))
            else:  # K_pad == 64: PE transpose per slot
                for j in range(spt):
                    sl = slice(64 * j, 64 * j + 64)
                    aT_ps = ps.tile([64, 64], F32, tag="aT_ps")
                    nc.tensor.transpose(aT_ps[:], attn[sl, :], ident[:],
                                        tile_position=(64 * j, 0))
                    aT_sb = small.tile([64, 64], F32, tag="aT_sb")
                    nc.scalar.activation(aT_sb[:], aT_ps[:],
                                         mybir.ActivationFunctionType.Identity)
                    nc.tensor.matmul(s_ps[sl, :], aT_sb[:], hk_t[sl, :],
                                     start=True, stop=True,
                                     tile_position=(0, 64 * j))

            s_sb = small.tile([128, D], F32, tag="s_sb")
            nc.scalar.activation(s_sb[:], s_ps[:],
                                 mybir.ActivationFunctionType.Identity,
                                 scale=rinv[:])
            nc.sync.dma_start(out[r0:r0 + 128, :], s_sb[:])

    nc.compile()
    return nc


def _plan(seg):
    T = seg.shape[0]
    change = np.nonzero(np.diff(seg))[0] + 1
    starts = np.concatenate([[0], change]).astype(np.int64)
    ends = np.concatenate([change, [T]]).astype(np.int64)
    sizes = ends - starts
    smax = int(sizes.max())
    if smax <= 32:
        K_pad = 32
    elif smax <= 64:
        K_pad = 64
    else:
        raise NotImplementedError(f"group size {smax} > 64")
    G = len(starts)
    S_dev = -(-G // N_CORES)
    rows = -(-(S_dev * K_pad) // 128) * 128
    spt = 128 // K_pad
    n_tiles = rows // 128

    # size-descending boustrophedon assignment: rank r -> core, slot r//8
    order = np.argsort(-sizes, kind="stable")          # group ids by size desc
    assign = {}                                        # g -> (core, slot)
    for r, g in enumerate(order):
        j = r // N_CORES
        c = r % N_CORES if j % 2 == 0 else N_CORES - 1 - (r % N_CORES)
        assign[int(g)] = (c, j)
    sizes_desc = sizes[order]
    K_tile = []
    for t in range(n_tiles):
        r = t * spt * N_CORES
        K_tile.append(int(sizes_desc[r]) if r < G else 1)
    return starts, ends, sizes, G, K_pad, S_dev, rows, assign, tuple(K_tile)


def _pack(f, h, seg, W, b):
    starts, ends, sizes, G, K_pad, S_dev, rows, assign, K_tile = _plan(seg)
    wt_aug = np.concatenate([W.T, b[None, :]], axis=0)  # [65, 64]
    ident = np.eye(64, dtype=np.float32)

    fpackt = np.zeros((N_CORES, rows, K_pad * D), dtype=np.float32)
    hkey = np.zeros((N_CORES, rows, D), dtype=np.float32)
    hkt_aug = np.zeros((N_CORES, D + 1, rows), dtype=np.float32)
    hkt_aug[:, D, :] = 1.0
    m0 = np.full((N_CORES, rows, K_pad), NEG, dtype=np.float32)
    for g in range(G):
        c, j = assign[g]
        lo, hi, s = starts[g], ends[g], int(sizes[g])
        r = j * K_pad
        blk = f[lo:hi, lo:hi, :]                      # [q, k, d]
        fpackt[c, r:r + s, :s * D] = blk.transpose(1, 0, 2).reshape(s, s * D)
        hkey[c, r:r + s, :] = h[lo:hi, :]
        hkt_aug[c, :D, r:r + s] = h[lo:hi, :].T
        m0[c, r:r + s, :s] = 0.0
        m0[c, np.arange(r, r + s), np.arange(s)] = SELF_MASK
    in_maps = [{"fpackt": fpackt[c], "hkey": hkey[c], "hkt_aug": hkt_aug[c],
                "wt_aug": wt_aug, "m0": m0[c], "ident": ident}
               for c in range(N_CORES)]
    meta = (starts, ends, sizes, G, K_pad, S_dev, rows, assign, K_tile)
    return in_maps, meta


def _unpack(per_core_out, meta, T):
    starts, ends, sizes, G, K_pad, S_dev, rows, assign, K_tile = meta
    outf = np.zeros((T, D), dtype=np.float32)
    for g in range(G):
        c, j = assign[g]
        if sizes[g] > 1:
            outf[starts[g]:ends[g], :] = \
                per_core_out[c][j * K_pad:j * K_pad + int(sizes[g]), :]
    return outf


def kernel(f, h, segment_ids, W, b):
    global LAST_RESULT
    f = np.asarray(f, dtype=np.float32)
    h = np.asarray(h, dtype=np.float32)
    seg = np.asarray(segment_ids)
    W = np.asarray(W, dtype=np.float32)
    b = np.asarray(b, dtype=np.float32)
    T = h.shape[0]

    in_maps, meta = _pack(f, h, seg, W, b)
    K_pad, rows, K_tile = meta[4], meta[6], meta[8]

    key = (K_pad, rows, K_tile)
    if key not in _PROGRAM_CACHE:
        _PROGRAM_CACHE[key] = _build_program(K_pad, rows, K_tile)
    nc = _PROGRAM_CACHE[key]

    res = run_bass_kernel_spmd(nc, in_maps, core_ids=list(range(N_CORES)))
    LAST_RESULT = res
    return _unpack([res.results[dev]["out"] for dev in range(N_CORES)], meta, T)
